# revision 11
# baseline (speedup 1.0000x reference)
"""Distributed Bass kernel for sparse cluster attention on 8 TRN2 NeuronCores.

Sharding: tensor-parallel over heads (16 heads -> 2 per core).

Host->device traffic is the dominant cost of one execution (inputs stream
through the axon tunnel at ~10 GB/s every run), so all core-replicated
inputs are sharded on the host and AllGather'd on device:
  packin [PACKN] bf16 = [ xT token-slice | keyframe-lo col-slice | wproj
  row-slice ] -> one AllGather -> agp_out [8, PACKN] readable by frame.
Per-core-distinct inputs (wqkv/wqk_l head slices, biases) upload directly.
The output returns as fp16 (half the bytes of f32; |out|max ~ 0.4).

Per core:
  1. fp32-accurate keyframe q/k (hi/lo bf16 split) -> attn_score partial,
     AllReduce(max) over cores.
  2. main qkv in bf16: qT [ch,tok] kept in SBUF; k,v staged to DRAM [tok,ch].
  3. on-device top-153 per cluster: rank via comparison matrix, patch ids via
     one-hot matmul, gather index lists staged through DRAM.
  4. dma_gather of k (transposed) and v per consumer cluster; flash-style
     attention (logits MM -> exp on ACT -> av MM with ones-augmented v for
     the softmax denominator).
  5. AllToAll of per-core attention output [128, N] -> proj on this core's
     token slice -> out [2048, 1024] f16; host concatenates.
"""

import numpy as np
import ml_dtypes

import os
import concourse.bass as bass
import concourse.bacc as bacc
import concourse.mybir as mybir
import concourse.tile as tile
from concourse.bass_utils import run_bass_kernel_spmd

BF16 = mybir.dt.bfloat16
F16 = mybir.dt.float16
F32 = mybir.dt.float32
I16 = mybir.dt.int16
I32 = mybir.dt.int32
AF = mybir.ActivationFunctionType
OP = mybir.AluOpType

# problem constants
H, D, C = 16, 64, 1024
S, P = 32, 512
K, FC = 4, 8
N = S * P                      # 16384 tokens
TK = 153                       # top-k patches per cluster
NSUB = 5                       # subsampled frames
NCORES = 8
HC = H // NCORES               # heads per core = 2
CHC = HC * D                   # channels per core = 128
TOKS = N // NCORES             # output tokens per core = 2048
SCALE = float(D) ** -0.5

# packed AllGather layout (bf16 elements, per core)
KFTOK = K * P                  # keyframe tokens = 2048
KFSH = KFTOK // NCORES         # keyframe-lo cols per core = 256
XOFF = 0                       # xT slice [C, TOKS]
XKOFF = C * TOKS               # keyframe-lo slice [C, KFSH]
WPOFF = XKOFF + C * KFSH       # wproj row slice [128, C]
PACKN = WPOFF + 128 * C

_CACHE: dict = {}


def _blocks_for(ci, clusters):
    """kv gather blocks for consumer cluster ci: (src, frames, valid, Jb)."""
    blocks = []
    for src in range(K):
        nf = FC if src in (0, ci) else NSUB
        frames = [int(f) for f in clusters[src][:nf]]
        valid = TK * nf
        jb = ((valid + 127) // 128) * 128
        blocks.append((src, frames, valid, jb))
    return blocks


def build_nc(clusters, keyframes):
    NOAR = os.environ.get("KNOAR", "0") == "1"
    NOGATHER = os.environ.get("KNOGATHER", "0") == "1"
    NOA2A = os.environ.get("KNOA2A", "0") == "1"
    STUB = os.environ.get("KSTUB", "0") == "1"
    nc = bacc.Bacc(None, target_bir_lowering=False, debug=False)

    # ---- kernel I/O (per-core shards prepared on host) ----
    packin = nc.dram_tensor("packin", [PACKN], BF16, kind="ExternalInput")
    wqkv = nc.dram_tensor("wqkv", [C, 3 * CHC], BF16, kind="ExternalInput")
    bqkv = nc.dram_tensor("bqkv", [3 * CHC], F32, kind="ExternalInput")
    wqk_l = nc.dram_tensor("wqk_l", [C, 2 * CHC], BF16, kind="ExternalInput")
    bproj = nc.dram_tensor("bproj", [C], F32, kind="ExternalInput")
    out_ext = nc.dram_tensor("out", [TOKS, C], F16, kind="ExternalOutput")

    # ---- internal DRAM ----
    agp_in = nc.dram_tensor("agp_in", [PACKN], BF16)
    agp_out = nc.dram_tensor("agp_out", [NCORES, PACKN], BF16, addr_space="Shared")
    kD = nc.dram_tensor("kD", [N, CHC], BF16)
    vD = nc.dram_tensor("vD", [N, CHC], BF16)
    sc_in = nc.dram_tensor("sc_in", [K * P], F32)
    sc_out = nc.dram_tensor("sc_out", [K * P], F32, addr_space="Shared")
    ag_in = nc.dram_tensor("ag_in", [NCORES, CHC, TOKS], BF16)
    ag_out = nc.dram_tensor("ag_out", [NCORES, CHC, TOKS], BF16)
    idx_d = [nc.dram_tensor(f"idx_d{ci}", [4224], I16) for ci in range(K)]

    KFT = K * P  # keyframe tokens = 2048
    kf = [int(f) for f in keyframes]

    def x_tile_ap(frame, cc):
        """AP of x^T [128ch, 512tok] for chunk cc of a frame, from agp_out."""
        j, col0 = frame // 4, (frame % 4) * 512
        return (agp_out.ap()[j:j + 1, XOFF:XOFF + C * TOKS]
                .rearrange("a (p c) -> (a p) c", p=C)
                [cc * 128:(cc + 1) * 128, col0:col0 + 512])

    def xkfl_tile_ap(j, cc):
        """AP of keyframe-lo x^T [128ch, 256tok] for global col slice j."""
        return (agp_out.ap()[j:j + 1, XKOFF:XKOFF + C * KFSH]
                .rearrange("a (p c) -> (a p) c", p=C)
                [cc * 128:(cc + 1) * 128, 0:KFSH])

    if STUB:
        with tile.TileContext(nc) as tc:
            with tc.tile_pool(name="sp", bufs=2) as sp:
                t = sp.tile([128, 512], BF16)
                nc.sync.dma_start(t[:], packin.ap()[0:128 * 512].rearrange("(p c) -> p c", p=128))
                t2 = sp.tile([128, 512], F16)
                nc.vector.tensor_copy(t2[:], t[:])
                nc.sync.dma_start(out_ext.ap()[0:128, 0:512], t2[:])
        nc.finalize()
        return nc

    with tile.TileContext(nc) as tc:
        with (
            tc.tile_pool(name="persist", bufs=1) as pp,
            tc.tile_pool(name="work", bufs=3) as wp,
            tc.tile_pool(name="xp", bufs=16) as xp,
            tc.tile_pool(name="kvstage", bufs=3) as kvp,
            tc.tile_pool(name="expw", bufs=2) as ep,
            tc.tile_pool(name="gath", bufs=1) as gp,
            tc.tile_pool(name="psmed", bufs=2, space="PSUM") as psM,
            tc.tile_pool(name="psav", bufs=2, space="PSUM") as psV,
            tc.tile_pool(name="psbig", bufs=1, space="PSUM") as psL,
        ):
            # ================= input AllGather =================
            nc.sync.dma_start(agp_in.ap(), packin.ap())
            nc.gpsimd.collective_compute(
                "AllGather", OP.bypass,
                replica_groups=[list(range(NCORES))],
                ins=[agp_in.ap().opt()],
                outs=[agp_out.ap().opt()],
            )

            # ================= persistent SBUF =================
            qT = pp.tile([CHC, N], BF16, tag="qT")            # 4 MB
            attnT = pp.tile([CHC, N], BF16, tag="attnT")      # 4 MB
            ones_rowb = pp.tile([1, 128], BF16, tag="onesb")
            nc.vector.memset(ones_rowb[:], 1.0)
            onesf_row = pp.tile([1, 128], F32, tag="onesf")
            nc.vector.memset(onesf_row[:], 1.0)
            onesf_col = pp.tile([128, 1], F32, tag="onesfc")
            nc.vector.memset(onesf_col[:], 1.0)

            # weight tiles
            wqkv_t = pp.tile([128, 8, 3 * CHC], BF16, tag="wqkv")
            nc.sync.dma_start(wqkv_t[:], wqkv.ap().rearrange("(a p) c -> p a c", p=128))
            wqkl_t = pp.tile([128, 8, 2 * CHC], BF16, tag="wqkl")
            nc.sync.dma_start(wqkl_t[:], wqk_l.ap().rearrange("(a p) c -> p a c", p=128))

            # bias columns (per-partition layout)
            bq_col = pp.tile([128, 1], F32, tag="bqcol")
            nc.sync.dma_start(bq_col[:], bqkv.ap()[0:CHC].rearrange("(p a) -> p a", a=1))
            bkv_row = pp.tile([1, 2 * CHC], F32, tag="bkvrow")
            nc.sync.dma_start(bkv_row[:], bqkv.ap()[CHC:3 * CHC].rearrange("(a c) -> a c", a=1))
            bkv_row_b = pp.tile([1, 2 * CHC], BF16, tag="bkvrowb")
            nc.vector.tensor_copy(bkv_row_b[:], bkv_row[:])
            bqk_k = pp.tile([128, 1], F32, tag="bqkk")
            nc.sync.dma_start(bqk_k[:], bqkv.ap()[CHC:2 * CHC].rearrange("(p a) -> p a", a=1))

            # ================= phase 1: keyframe scores (fp32 accurate) ======
            qkf = gp.tile([128, KFT], F32, tag="kselT", name="qkf")
            kkf = gp.tile([128, KFT], F32, tag="vsel", name="kkf")
            for tt in range(KFT // 512):
                xh = [xp.tile([128, 512], BF16, tag="xmain", name=f"xh{tt}_{i}") for i in range(8)]
                xl = [xp.tile([128, 512], BF16, tag="xmain", name=f"xl{tt}_{i}") for i in range(8)]
                for cc in range(8):
                    nc.sync.dma_start(xh[cc][:], x_tile_ap(kf[tt], cc))
                    for half in range(2):
                        nc.sync.dma_start(xl[cc][:, half * KFSH:(half + 1) * KFSH],
                                          xkfl_tile_ap(tt * 2 + half, cc))
                for ot, (dst, bias) in enumerate(((qkf, bq_col), (kkf, bqk_k))):
                    ps = psM.tile([128, 512], F32, tag="med")
                    nmm = 8 * 3
                    i = 0
                    for cc in range(8):
                        w_h = wqkv_t[:, cc, ot * CHC:(ot + 1) * CHC]
                        w_l = wqkl_t[:, cc, ot * CHC:(ot + 1) * CHC]
                        for (wt, xt) in ((w_h, xh[cc]), (w_h, xl[cc]), (w_l, xh[cc])):
                            nc.tensor.matmul(ps[:], wt, xt[:], start=(i == 0), stop=(i == nmm - 1))
                            i += 1
                    nc.vector.tensor_scalar(dst[:, tt * 512:(tt + 1) * 512], ps[:], bias[:], None, OP.add)

            # prod + per-head reduce + max over the 2 local heads
            nc.vector.tensor_tensor(qkf[:], qkf[:], kkf[:], OP.mult)  # qkf <- q*k
            for ntile in range(KFT // 512):
                sl = slice(ntile * 512, (ntile + 1) * 512)
                ps0 = psM.tile([1, 512], F32, tag="med")
                ps1 = psM.tile([1, 512], F32, tag="med")
                nc.tensor.matmul(ps0[:], onesf_col[0:64, :], qkf[0:64, sl], start=True, stop=True)
                nc.tensor.matmul(ps1[:], onesf_col[64:128, :], qkf[64:128, sl], start=True, stop=True)
                s1sb = wp.tile([1, 512], F32, tag="s1sb", bufs=1)
                nc.vector.tensor_copy(s1sb[:], ps1[:])
                smax_t = wp.tile([1, 512], F32, tag="smax", bufs=2)
                nc.vector.tensor_tensor(smax_t[:], ps0[:], s1sb[:], OP.max)
                nc.sync.dma_start(
                    sc_in.ap()[ntile * 512:(ntile + 1) * 512].rearrange("(a c) -> a c", a=1),
                    smax_t[:])
            if NOAR:
                nc.sync.dma_start(sc_out.ap(), sc_in.ap())
            else:
                nc.gpsimd.collective_compute(
                    "AllReduce", OP.max,
                    replica_groups=[list(range(NCORES))],
                    ins=[sc_in.ap().opt()],
                    outs=[sc_out.ap().opt()],
                )

            # ================= phase 2: main qkv (bf16) =================
            for tt in range(N // 512):
                xt = [xp.tile([128, 512], BF16, tag="xmain", name=f"xt{tt}_{i}") for i in range(8)]
                for cc in range(8):
                    nc.sync.dma_start(xt[cc][:], x_tile_ap(tt, cc))
                # q: [ch, tok]
                psq = psM.tile([128, 512], F32, tag="med")
                for cc in range(8):
                    nc.tensor.matmul(psq[:], wqkv_t[:, cc, 0:CHC], xt[cc][:],
                                     start=(cc == 0), stop=(cc == 7))
                nc.vector.tensor_scalar(qT[:, tt * 512:(tt + 1) * 512], psq[:], bq_col[:], None, OP.add)
                # k,v: [tok, ch] staged to DRAM
                kvstage = kvp.tile([128, 4, 2 * CHC], BF16, tag="kvst")
                for sub in range(4):
                    pskv = psM.tile([128, 2 * CHC], F32, tag="med")
                    for cc in range(8):
                        nc.tensor.matmul(pskv[:], xt[cc][:, sub * 128:(sub + 1) * 128],
                                         wqkv_t[:, cc, CHC:3 * CHC],
                                         start=(cc == 0), stop=False)
                    nc.tensor.matmul(pskv[:], ones_rowb[:], bkv_row_b[:],
                                     start=False, stop=True)
                    nc.vector.tensor_copy(kvstage[:, sub, :], pskv[:])
                tok0 = tt * 512
                nc.sync.dma_start(
                    kD.ap()[tok0:tok0 + 512, :].rearrange("(s p) c -> p s c", p=128),
                    kvstage[:, :, 0:CHC])
                nc.sync.dma_start(
                    vD.ap()[tok0:tok0 + 512, :].rearrange("(s p) c -> p s c", p=128),
                    kvstage[:, :, CHC:2 * CHC])

            # ================= phase 3: top-k selection =================
            iota_r1 = wp.tile([128, 128], I32, tag="ior1")
            nc.gpsimd.iota(iota_r1[:], pattern=[[1, 128]], base=0, channel_multiplier=0)
            iota_r1f = pp.tile([128, 128], F32, tag="ior1f")
            nc.vector.tensor_copy(iota_r1f[:], iota_r1[:])
            iota_r2 = wp.tile([128, 32], I32, tag="ior2")
            nc.gpsimd.iota(iota_r2[:], pattern=[[1, 32]], base=128, channel_multiplier=0)
            iota_r2f = pp.tile([128, 32], F32, tag="ior2f")
            nc.vector.tensor_copy(iota_r2f[:], iota_r2[:])
            iota_pv = wp.tile([128, 4], I32, tag="iopv")
            nc.gpsimd.iota(iota_pv[:], pattern=[[128, 4]], base=0, channel_multiplier=1)
            iota_pvf = pp.tile([128, 4], F32, tag="iopvf")
            nc.vector.tensor_copy(iota_pvf[:], iota_pv[:])
            zpad = pp.tile([1, 64], I16, tag="zpad")
            nc.vector.memset(zpad[:], 0)

            psel_sb = {}
            for cl in range(K):
                s_row = wp.tile([1, P], F32, tag="srow")
                nc.sync.dma_start(s_row[:], sc_out.ap()[cl * P:(cl + 1) * P].rearrange("(a c) -> a c", a=1))
                s_colT = wp.tile([128, 4], F32, tag="scolT")
                nc.sync.dma_start(
                    s_colT[:], sc_out.ap()[cl * P:(cl + 1) * P].rearrange("(a p) -> p a", p=128))
                ps_bc = psM.tile([128, P], F32, tag="med")
                nc.tensor.matmul(ps_bc[:], onesf_row[:], s_row[:], start=True, stop=True)
                s_bc = wp.tile([128, P], F32, tag="sbc", bufs=2)
                nc.vector.tensor_copy(s_bc[:], ps_bc[:])
                ps_p1 = psM.tile([128, 1], F32, tag="med")
                ps_p2 = psM.tile([32, 1], F32, tag="med")
                for pc in range(4):
                    gt = wp.tile([128, P], F32, tag="gtm", bufs=2)
                    nc.vector.tensor_scalar(gt[:], s_bc[:], s_colT[:, pc:pc + 1], None, OP.is_gt)
                    rank = wp.tile([128, 1], F32, tag="rank")
                    nc.vector.reduce_sum(rank[:], gt[:], axis=mybir.AxisListType.X)
                    eq1 = wp.tile([128, 128], F32, tag="eq1")
                    nc.vector.tensor_scalar(eq1[:], iota_r1f[:], rank[:], None, OP.is_equal)
                    eq2 = wp.tile([128, 32], F32, tag="eq2")
                    nc.vector.tensor_scalar(eq2[:], iota_r2f[:], rank[:], None, OP.is_equal)
                    nc.tensor.matmul(ps_p1[:], eq1[:], iota_pvf[:, pc:pc + 1],
                                     start=(pc == 0), stop=(pc == 3))
                    nc.tensor.matmul(ps_p2[:], eq2[:], iota_pvf[:, pc:pc + 1],
                                     start=(pc == 0), stop=(pc == 3))
                p1 = pp.tile([128, 1], F32, tag=f"psel1_{cl}")
                nc.vector.tensor_copy(p1[:], ps_p1[:])
                p2 = pp.tile([32, 1], F32, tag=f"psel2_{cl}")
                nc.vector.tensor_copy(p2[:], ps_p2[:])
                psel_sb[cl] = (p1, p2)

            # build gather index lists in DRAM, then wrapped re-read
            idx_sb = {}
            for ci in range(K):
                blocks = _blocks_for(ci, clusters)
                base = 0
                for (src, frames, valid, jb) in blocks:
                    nf = len(frames)
                    p1, p2 = psel_sb[src]
                    t1 = wp.tile([128, nf], F32, tag="tok1")
                    for fi, f in enumerate(frames):
                        nc.vector.tensor_scalar(t1[:, fi:fi + 1], p1[:], float(f * P), None, OP.add)
                    t1i = wp.tile([128, nf], I32, tag="tok1i")
                    nc.vector.tensor_copy(t1i[:], t1[:])
                    t1s = wp.tile([128, nf], I16, tag="tok1s")
                    nc.vector.tensor_copy(t1s[:], t1i[:])
                    nc.sync.dma_start(
                        idx_d[ci].ap()[base:base + 128 * nf].rearrange("(p f) -> p f", f=nf),
                        t1s[:])
                    rows = min(32, jb // nf - 128)
                    t2 = wp.tile([32, nf], F32, tag="tok2")
                    for fi, f in enumerate(frames):
                        nc.vector.tensor_scalar(t2[0:rows, fi:fi + 1], p2[0:rows, :], float(f * P), None, OP.add)
                    t2i = wp.tile([32, nf], I32, tag="tok2i")
                    nc.vector.tensor_copy(t2i[0:rows, :], t2[0:rows, :])
                    t2s = wp.tile([32, nf], I16, tag="tok2s")
                    nc.vector.tensor_copy(t2s[0:rows, :], t2i[0:rows, :])
                    written = 128 * nf + rows * nf
                    nc.sync.dma_start(
                        idx_d[ci].ap()[base + 128 * nf:base + written].rearrange("(p f) -> p f", f=nf),
                        t2s[0:rows, :])
                    if written < jb:
                        nc.sync.dma_start(
                            idx_d[ci].ap()[base + written:base + jb].rearrange("(a c) -> a c", a=1),
                            zpad[:, 0:jb - written])
                    base += jb
                jtot = base
                isb = gp.tile([128, jtot // 16], I16, tag="idxsb", bufs=4, name=f"isb{ci}")
                for rep in range(8):
                    nc.sync.dma_start(
                        isb[rep * 16:(rep + 1) * 16, :],
                        idx_d[ci].ap()[0:jtot].rearrange("(s p) -> p s", p=16))
                idx_sb[ci] = (isb, jtot)

            # ================= phase 4: attention per cluster =================
            for ci in range(K):
                blocks = _blocks_for(ci, clusters)
                isb, jtot = idx_sb[ci]
                nch = jtot // 128
                kselT = gp.tile([128, 1, jtot], BF16, tag="kselT")
                vsel = gp.tile([128, nch, CHC], BF16, tag="vsel")
                if NOGATHER:
                    nc.vector.memset(kselT[:], 0.01)
                    nc.vector.memset(vsel[:], 0.01)
                else:
                    for g in range(jtot // 512):
                        nc.gpsimd.dma_gather(
                            kselT[:, :, g * 512:(g + 1) * 512], kD.ap(),
                            isb[:, g * 32:(g + 1) * 32],
                            num_idxs=512, num_idxs_reg=512,
                            elem_size=CHC, transpose=True)
                        nc.gpsimd.dma_gather(
                            vsel[:, g * 4:(g + 1) * 4, :], vD.ap(),
                            isb[:, g * 32:(g + 1) * 32],
                            num_idxs=512, num_idxs_reg=512,
                            elem_size=CHC, transpose=False)
                # v_aug: [128, nch, 130]: h0 v | h0 ones | h1 v | h1 ones
                vaug = gp.tile([128, nch, 130], BF16, tag="vaug")
                nc.vector.memset(vaug[:], 0.0)
                base = 0
                for (src, frames, valid, jb) in blocks:
                    c0 = base // 128
                    nfull = valid // 128
                    rem = valid - nfull * 128
                    for h in range(2):
                        if nfull > 0:
                            nc.vector.tensor_copy(
                                vaug[:, c0:c0 + nfull, h * 65:h * 65 + 64],
                                vsel[:, c0:c0 + nfull, h * 64:(h + 1) * 64])
                            nc.vector.memset(vaug[:, c0:c0 + nfull, h * 65 + 64:h * 65 + 65], 1.0)
                        if rem > 0:
                            nc.vector.tensor_copy(
                                vaug[0:rem, c0 + nfull, h * 65:h * 65 + 64],
                                vsel[0:rem, c0 + nfull, h * 64:(h + 1) * 64])
                            nc.vector.memset(vaug[0:rem, c0 + nfull, h * 65 + 64:h * 65 + 65], 1.0)
                    base += jb

                for qt in range(FC):
                    f_q = int(clusters[ci][qt])
                    qsl = slice(f_q * P, (f_q + 1) * P)
                    ps_av = [psV.tile([65, 512], F32, tag="av", name=f"psav{ci}_{qt}_{i}") for i in range(2)]
                    ngrp = nch // 2
                    for g in range(ngrp):
                        ps_lg = psL.tile([128, 2048], F32, tag="big")
                        for c01 in range(2):
                            c = g * 2 + c01
                            for h in range(2):
                                nc.tensor.matmul(
                                    ps_lg[:, (c01 * 2 + h) * 512:(c01 * 2 + h + 1) * 512],
                                    kselT[h * 64:(h + 1) * 64, 0, c * 128:(c + 1) * 128],
                                    qT[h * 64:(h + 1) * 64, qsl],
                                    start=True, stop=True,
                                    tile_position=(h * 64, 0))
                        ew = ep.tile([128, 2048], BF16, tag="ew")
                        nc.scalar.activation(ew[:], ps_lg[:], AF.Exp, scale=SCALE)
                        for c01 in range(2):
                            c = g * 2 + c01
                            for h in range(2):
                                nc.tensor.matmul(
                                    ps_av[h][:],
                                    vaug[:, c, h * 65:(h + 1) * 65],
                                    ew[:, (c01 * 2 + h) * 512:(c01 * 2 + h + 1) * 512],
                                    start=(g == 0 and c01 == 0),
                                    stop=(g == ngrp - 1 and c01 == 1))
                    for h in range(2):
                        rec = wp.tile([1, 512], F32, tag="rec")
                        nc.vector.reciprocal(rec[:], ps_av[h][64:65, :])
                        ps_bc2 = psM.tile([64, 512], F32, tag="med")
                        nc.tensor.matmul(ps_bc2[:], onesf_row[:, 0:64], rec[:],
                                         start=True, stop=True)
                        bc_sb = wp.tile([64, 512], F32, tag="bcsb", bufs=2)
                        nc.vector.tensor_copy(bc_sb[:], ps_bc2[:])
                        nc.vector.tensor_tensor(
                            attnT[h * 64:(h + 1) * 64, qsl],
                            ps_av[h][0:64, :], bc_sb[:], OP.mult)

            # ================= phase 5: AllToAll + proj =================
            nc.sync.dma_start(
                ag_in.ap().rearrange("j p t -> p j t"),
                attnT[:].rearrange("p (j t) -> p j t", j=NCORES))
            if NOA2A:
                nc.sync.dma_start(ag_out.ap(), ag_in.ap())
            else:
                nc.gpsimd.collective_compute(
                    "AllToAll", OP.bypass,
                    replica_groups=[list(range(NCORES))],
                    ins=[ag_in.ap().opt()],
                    outs=[ag_out.ap().opt()],
                )
            wpj = pp.tile([128, 8, C], BF16, tag="wpj")
            nc.sync.dma_start(
                wpj[:],
                agp_out.ap()[0:NCORES, WPOFF:WPOFF + 128 * C]
                .rearrange("j (p c) -> p j c", p=128))
            bpj_row = pp.tile([1, C], BF16, tag="bpj")
            bpj_f = wp.tile([1, C], F32, tag="bpjf")
            nc.sync.dma_start(bpj_f[:], bproj.ap().rearrange("(a c) -> a c", a=1))
            nc.vector.tensor_copy(bpj_row[:], bpj_f[:])
            atk2 = pp.tile([128, 8, TOKS], BF16, tag="attnT", name="atk2")
            nc.sync.dma_start(atk2[:], ag_out.ap().rearrange("j p t -> p j t"))
            for mt in range(TOKS // 128):
                for ntile in range(2):
                    nsl = slice(ntile * 512, (ntile + 1) * 512)
                    ps = psM.tile([128, 512], F32, tag="med")
                    for cc in range(8):
                        nc.tensor.matmul(ps[:], atk2[:, cc, mt * 128:(mt + 1) * 128],
                                         wpj[:, cc, nsl], start=(cc == 0), stop=False)
                    nc.tensor.matmul(ps[:], ones_rowb[:], bpj_row[:, nsl],
                                     start=False, stop=True)
                    ot = wp.tile([128, 512], F16, tag="otile", bufs=2)
                    nc.vector.tensor_copy(ot[:], ps[:])
                    nc.sync.dma_start(
                        out_ext.ap()[mt * 128:(mt + 1) * 128, nsl], ot[:])

    nc.finalize()
    return nc


def _host_prep(x, W_qkv, b_qkv, W_proj, b_proj, clusters, keyframes):
    bf = ml_dtypes.bfloat16
    x2 = np.ascontiguousarray(x.reshape(N, C))
    xbT = np.ascontiguousarray(x2.T.astype(bf))                       # [C, N]
    kf_tok = np.concatenate([np.arange(P, dtype=np.int64) + int(f) * P for f in keyframes])
    xkf = x2[kf_tok]                                                   # [2048, C] f32
    xkf_h = xkf.astype(bf)
    xkf_l = (xkf - xkf_h.astype(np.float32)).astype(bf)
    xkfT_l = np.ascontiguousarray(xkf_l.T)                             # [C, 2048]
    wproj_b = W_proj.astype(bf)                                        # [C, C]

    in_maps = []
    for core in range(NCORES):
        h0 = core * HC
        qcols = np.arange(h0 * D, (h0 + HC) * D)
        wq = W_qkv[:, qcols]
        wk = W_qkv[:, C + qcols]
        wv = W_qkv[:, 2 * C + qcols]
        wqkv_s = np.concatenate([wq, wk, wv], axis=1)                  # [C, 384]
        bq = b_qkv[qcols]
        bk = b_qkv[C + qcols]
        bv = b_qkv[2 * C + qcols]
        wqk = np.concatenate([wq, wk], axis=1)                         # [C, 256]
        wqk_hi = wqk.astype(bf)
        wqk_lo = (wqk - wqk_hi.astype(np.float32)).astype(bf)
        packin = np.concatenate([
            xbT[:, core * TOKS:(core + 1) * TOKS].ravel(),
            xkfT_l[:, core * KFSH:(core + 1) * KFSH].ravel(),
            wproj_b[core * 128:(core + 1) * 128, :].ravel(),
        ])
        assert packin.shape[0] == PACKN
        in_maps.append({
            "packin": np.ascontiguousarray(packin),
            "wqkv": np.ascontiguousarray(wqkv_s.astype(bf)),
            "bqkv": np.ascontiguousarray(np.concatenate([bq, bk, bv]).astype(np.float32)),
            "wqk_l": np.ascontiguousarray(wqk_lo),
            "bproj": np.ascontiguousarray(b_proj.astype(np.float32)),
        })
    return in_maps


def kernel(x, W_qkv, b_qkv, W_proj, b_proj, clusters, keyframes, **run_kwargs):
    x = np.asarray(x, dtype=np.float32)
    W_qkv = np.asarray(W_qkv, dtype=np.float32)
    b_qkv = np.asarray(b_qkv, dtype=np.float32)
    W_proj = np.asarray(W_proj, dtype=np.float32)
    b_proj = np.asarray(b_proj, dtype=np.float32)
    clusters = np.asarray(clusters, dtype=np.int32)
    keyframes = np.asarray(keyframes, dtype=np.int32)

    key = (clusters.tobytes(), keyframes.tobytes(), os.environ.get("KNOAR"),
           os.environ.get("KNOGATHER"), os.environ.get("KNOA2A"), os.environ.get("KSTUB"))
    if _CACHE.get("key") != key:
        _CACHE["nc"] = build_nc(clusters, keyframes)
        _CACHE["key"] = key
    nc = _CACHE["nc"]

    in_maps = _host_prep(x, W_qkv, b_qkv, W_proj, b_proj, clusters, keyframes)
    res = run_bass_kernel_spmd(nc, in_maps, core_ids=list(range(NCORES)), **run_kwargs)
    _CACHE["last_result"] = res
    outs = res.results
    full = np.concatenate([np.asarray(outs[c]["out"], dtype=np.float32) for c in range(NCORES)], axis=0)
    return full.reshape(1, N, C)


def bench(x, W_qkv, b_qkv, W_proj, b_proj, clusters, keyframes, iters=10, reps=5):
    """Steady-state on-device timing: chains `iters` NEFF executions inside one
    jit (data-dependency chained via bqkv), times the best of `reps` calls."""
    import time
    import jax
    import jax.numpy as jnp
    from jax.sharding import Mesh, PartitionSpec
    from jax.experimental.shard_map import shard_map
    from concourse import bass2jax
    from concourse.bass2jax import _bass_exec_p
    import concourse.mybir as _mb

    clusters = np.asarray(clusters, dtype=np.int32)
    keyframes = np.asarray(keyframes, dtype=np.int32)
    key = (clusters.tobytes(), keyframes.tobytes(), os.environ.get("KNOAR"),
           os.environ.get("KNOGATHER"), os.environ.get("KNOA2A"), os.environ.get("KSTUB"))
    if _CACHE.get("key") != key:
        _CACHE["nc"] = build_nc(clusters, keyframes)
        _CACHE["key"] = key
    nc = _CACHE["nc"]
    bass2jax.install_neuronx_cc_hook()

    in_maps = _host_prep(np.asarray(x, np.float32), np.asarray(W_qkv, np.float32),
                         np.asarray(b_qkv, np.float32), np.asarray(W_proj, np.float32),
                         np.asarray(b_proj, np.float32), clusters,
                         np.asarray(keyframes, np.int32))

    in_names, out_names, out_avals, zero_outs = [], [], [], []
    partition_name = nc.partition_id_tensor.name if nc.partition_id_tensor else None
    for alloc in nc.m.functions[0].allocations:
        if not isinstance(alloc, _mb.MemoryLocationSet):
            continue
        name = alloc.memorylocations[0].name
        if alloc.kind == "ExternalInput":
            if name != partition_name:
                in_names.append(name)
        elif alloc.kind == "ExternalOutput":
            out_names.append(name)
            shape = tuple(alloc.tensor_shape)
            dtype = _mb.dt.np(alloc.dtype)
            out_avals.append(jax.core.ShapedArray(shape, dtype))
            zero_outs.append(np.zeros(shape, dtype))
    n_params = len(in_names)
    all_in_names = list(in_names) + list(out_names)
    if partition_name is not None:
        all_in_names.append(partition_name)
    bq_pos = in_names.index("bqkv")

    def _body(*args):
        ops = list(args)
        if partition_name is not None:
            ops = ops + [bass2jax.partition_id_tensor()]
        outs = _bass_exec_p.bind(
            *ops,
            out_avals=tuple(out_avals),
            in_names=tuple(all_in_names),
            out_names=tuple(out_names),
            lowering_input_output_aliases=(),
            sim_require_finite=True,
            sim_require_nnan=True,
            nc=nc,
        )
        return tuple(outs)

    devices = jax.devices()[:NCORES]
    mesh = Mesh(np.asarray(devices), ("core",))
    in_specs = (PartitionSpec("core"),) * (n_params + len(out_names))
    out_specs = (PartitionSpec("core"),) * len(out_names)
    f = jax.jit(shard_map(_body, mesh=mesh, in_specs=in_specs,
                          out_specs=out_specs, check_rep=False))
    concat_in = [np.concatenate([np.asarray(in_maps[c][n]) for c in range(NCORES)], axis=0)
                 for n in in_names]
    concat_zeros = [np.zeros((NCORES * z.shape[0], *z.shape[1:]), z.dtype) for z in zero_outs]
    args = [jax.device_put(a) for a in concat_in + concat_zeros]
    o = f(*args)
    jax.block_until_ready(o)
    times = []
    for _ in range(max(reps, 20)):
        t0 = time.perf_counter()
        o = f(*args)
        jax.block_until_ready(o)
        times.append(time.perf_counter() - t0)
    times.sort()
    return times[0] * 1e9, times


def bench_floor(reps=20):
    """Dispatch-floor: time a trivial 8-core NEFF (one 64KB copy)."""
    import time
    import jax
    from jax.sharding import Mesh, PartitionSpec
    from jax.experimental.shard_map import shard_map
    from concourse import bass2jax
    from concourse.bass2jax import _bass_exec_p
    import concourse.mybir as _mb
    import concourse.bacc as _bacc
    import concourse.tile as _tile

    if "floor_nc" not in _CACHE:
        nc = _bacc.Bacc(None, target_bir_lowering=False, debug=False)
        a = nc.dram_tensor("a", [128, 128], F32, kind="ExternalInput")
        b = nc.dram_tensor("b", [128, 128], F32, kind="ExternalOutput")
        with _tile.TileContext(nc) as tc:
            with tc.tile_pool(name="p", bufs=1) as p:
                t = p.tile([128, 128], F32)
                nc.sync.dma_start(t[:], a.ap())
                nc.sync.dma_start(b.ap(), t[:])
        nc.finalize()
        _CACHE["floor_nc"] = nc
    nc = _CACHE["floor_nc"]
    bass2jax.install_neuronx_cc_hook()
    partition_name = nc.partition_id_tensor.name if nc.partition_id_tensor else None
    in_names = ["a", "b"]
    if partition_name is not None:
        in_names.append(partition_name)
    out_avals = (jax.core.ShapedArray((128, 128), np.float32),)

    def _body(*args):
        ops = list(args)
        if partition_name is not None:
            ops = ops + [bass2jax.partition_id_tensor()]
        return tuple(_bass_exec_p.bind(
            *ops, out_avals=out_avals, in_names=tuple(in_names),
            out_names=("b",), lowering_input_output_aliases=(),
            sim_require_finite=True, sim_require_nnan=True, nc=nc))

    devices = jax.devices()[:NCORES]
    mesh = Mesh(np.asarray(devices), ("core",))
    f = jax.jit(shard_map(_body, mesh=mesh,
                          in_specs=(PartitionSpec("core"),) * 2,
                          out_specs=(PartitionSpec("core"),), check_rep=False))
    a = jax.device_put(np.zeros((NCORES * 128, 128), np.float32))
    z = jax.device_put(np.zeros((NCORES * 128, 128), np.float32))
    o = f(a, z); jax.block_until_ready(o)
    times = []
    for _ in range(reps):
        t0 = time.perf_counter()
        o = f(a, z)
        jax.block_until_ready(o)
        times.append(time.perf_counter() - t0)
    times.sort()
    return times[0] * 1e9

